# revision 25
# baseline (speedup 1.0000x reference)
"""GNN message passing (copy_src + segment_sum + Linear + ReLU) on 8 TRN2 cores.

Strategy (sharding_hint): partition dst nodes into 128-node windows; assign
windows to the 8 cores (balanced by edge-tile count, serpentine over windows
sorted by cost). Each core holds a full replica of the (split) feature table
in its HBM in bf16 and gathers the src rows of its edges with dma_gather (the
int16 index range forces a two-half table split at 25000). Aggregation happens
on-chip: for each 128-edge tile, a one-hot(dst_local) matrix is built with a
DVE is_equal against an iota row (all bf16), and PE matmuls accumulate
aggT[f, n] += msgs[e, f].T @ onehot[e, n] into fp32 PSUM per window. The node
update (Linear + bias + ReLU) runs per window: transform matmul with W^T
(bf16), K=1 ones-row matmul to add the bias, ACT ReLU, DMA out in fp32.
No collectives — the host splits edges and concatenates the per-core window
outputs. Gathers rotate over 4 SWDGE queues so descriptor-ring drain overlaps
generation.

Self-contained: shapes hardcoded for feature[50000,128], src/dst[640000],
W[128,128], b[128].
"""
import dataclasses
import os as _os

import ml_dtypes
import numpy as np

import concourse.bacc as bacc
import concourse.bass as bass
import concourse.tile as tile
from concourse import mybir
from concourse.bass_utils import run_bass_kernel_spmd

P = 128
N_NODES = 50000
N_EDGES = 640000
VHALF = 25000          # feature table split point (int16 gather index range)
NC = 8
G = (N_NODES + P - 1) // P          # 391 global windows
W_SLOTS = (G + NC - 1) // NC        # 49 window slots per core
BATCH_TILES = 64                    # max edge tiles per gather batch
NQ = 4                              # SWDGE queues for gather rotation
INIT_BATCHES = 0                    # msgs buffers are pre-memset; no init pads

F32 = mybir.dt.float32
BF16 = mybir.dt.bfloat16
I16 = mybir.dt.int16
BF = ml_dtypes.bfloat16


def _make_plan(src, dst):
    """Host-side partition of edges into (core, slot, tile) with static,
    core-uniform tile counts. Returns the static schedule + per-core arrays."""
    src = np.asarray(src, dtype=np.int64)
    dst = np.asarray(dst, dtype=np.int64)
    win = dst >> 7
    order = np.lexsort((src, win))          # by window, then src (HBM locality)
    src_s = src[order]
    dst_s = dst[order]
    win_s = win[order]

    cnt = np.bincount(win_s, minlength=G)
    cntA = np.bincount(win_s[src_s < VHALF], minlength=G)
    cntB = cnt - cntA
    start = np.concatenate([[0], np.cumsum(cnt)])[:G]
    tilesA = -(-cntA // P)
    tilesB = -(-cntB // P)

    # group windows into slots of 8 with matching (tilesA, tilesB) so the
    # per-slot max over cores (the static padded capacity) is tight; within a
    # slot, hand the biggest window to the least-loaded core.
    idx_sorted = sorted(range(G), key=lambda g: (-tilesA[g], -tilesB[g]))
    assign = -np.ones((NC, W_SLOTS), dtype=np.int64)
    core_load = np.zeros(NC, dtype=np.int64)
    for w in range(W_SLOTS):
        chunk = idx_sorted[w * NC:(w + 1) * NC]
        chunk = sorted(chunk, key=lambda g: -cnt[g])
        order = np.argsort(core_load, kind="stable")
        for g, c in zip(chunk, order):
            assign[c, w] = g
            core_load[c] += cnt[g]

    # static per-slot tile capacities and real index counts (shared across
    # cores; cStat = max real edge count over cores, so num_idxs_reg can be
    # core-uniform while trailing -1 padding is skipped by the SWDGE)
    KA = np.zeros(W_SLOTS, dtype=np.int64)
    KB = np.zeros(W_SLOTS, dtype=np.int64)
    cStatA = np.zeros(W_SLOTS, dtype=np.int64)
    cStatB = np.zeros(W_SLOTS, dtype=np.int64)
    for w in range(W_SLOTS):
        for c in range(NC):
            g = assign[c, w]
            if g >= 0:
                cStatA[w] = max(cStatA[w], cntA[g])
                cStatB[w] = max(cStatB[w], cntB[g])
        KA[w] = -(-cStatA[w] // P)
        KB[w] = -(-cStatB[w] // P)
        if KA[w] + KB[w] == 0:
            KA[w] = 1                      # dummy tile keeps PSUM initialized

    # greedy batches of slots, <= BATCH_TILES tiles each; the final slots get
    # small batches so the post-gather compute tail is short
    slot_tiles = [int(KA[w] + KB[w]) for w in range(W_SLOTS)]
    total_tiles = sum(slot_tiles)
    batches = []
    cur = []
    cur_tiles = 0
    done_tiles = 0
    for w in range(W_SLOTS):
        kw = slot_tiles[w]
        if (total_tiles - done_tiles) <= 48:
            cap = 16        # small final batches: short post-gather tail
        else:
            cap = BATCH_TILES
        if cur and cur_tiles + kw > cap:
            batches.append(cur)
            cur = []
            cur_tiles = 0
        cur.append(w)
        cur_tiles += kw
        done_tiles += kw
    if cur:
        batches.append(cur)

    TA_tot = int(KA.sum())
    TB_tot = int(KB.sum())
    T_tot = TA_tot + TB_tot

    # static schedule description (identical across cores); ci_off is the
    # f32-column offset of this batch's [A idx | B idx] region in the idx
    # tensor (per-batch packing so the idx DMA can be split for early start)
    sched = []
    t_base = 0
    a_base = 0
    b_base = 0
    ci_off = 0
    for slots in batches:
        ka_b = int(sum(KA[w] for w in slots))
        kb_b = int(sum(KB[w] for w in slots))
        wins = []
        pa = 0
        pb = 0
        for w in slots:
            cols = list(range(pa, pa + int(KA[w]))) + \
                   list(range(ka_b + pb, ka_b + pb + int(KB[w])))
            wins.append((w, cols))
            pa += int(KA[w])
            pb += int(KB[w])
        sched.append(dict(ka=ka_b, kb=kb_b, t_base=t_base,
                          a_base=a_base, b_base=b_base, ci_off=ci_off,
                          wins=wins))
        t_base += ka_b + kb_b
        a_base += ka_b
        b_base += kb_b
        ci_off += (ka_b + kb_b) * 4

    # per-core data arrays; batches >= INIT_BATCHES get trailing -1 index
    # padding per window-half (SWDGE skips it), earlier batches keep full
    # zero-padding so every msgs buffer region gets initialized once
    dstloc = np.full((NC, P, T_tot), -1.0, dtype=BF)
    idxA = np.zeros((NC, TA_tot * P), dtype=np.int16)
    idxB = np.zeros((NC, TB_tot * P), dtype=np.int16)
    for bi, bt in enumerate(sched):
        ka_b = bt["ka"]
        for w, cols in bt["wins"]:
            nA = len([c for c in cols if c < ka_b])
            for c in range(NC):
                g = assign[c, w]
                if g < 0:
                    continue
                e0 = start[g]
                ca, cb = int(cntA[g]), int(cntB[g])
                dl = (dst_s[e0:e0 + ca + cb] - (g << 7)).astype(BF)
                sv = src_s[e0:e0 + ca + cb]
                # A half
                if ca:
                    jpos = np.arange(ca)
                    dstloc[c, jpos % P, bt["t_base"] + cols[0] + jpos // P] = dl[:ca]
                    # logical gather position = (prefixA + tile)*P + lane
                    gpos = (bt["a_base"] + cols[0]) * P + jpos
                    idxA[c, gpos] = sv[:ca].astype(np.int16)
                # B half
                if cb:
                    jpos = np.arange(cb)
                    bcol0 = cols[nA] if cb else 0
                    dstloc[c, jpos % P, bt["t_base"] + bcol0 + jpos // P] = dl[ca:]
                    gpos = (bt["b_base"] + (bcol0 - ka_b)) * P + jpos
                    idxB[c, gpos] = (sv[ca:] - VHALF).astype(np.int16)
            if bi >= INIT_BATCHES and _os.environ.get("KTRIM", "1") != "0":
                if KA[w]:
                    p0 = (bt["a_base"] + cols[0]) * P
                    idxA[:, p0 + cStatA[w]:p0 + int(KA[w]) * P] = -1
                if KB[w]:
                    bcol0 = cols[nA]
                    p0 = (bt["b_base"] + (bcol0 - ka_b)) * P
                    idxB[:, p0 + cStatB[w]:p0 + int(KB[w]) * P] = -1

    return dict(sched=sched, assign=assign, KA=KA, KB=KB,
                cStatA=cStatA, cStatB=cStatB,
                TA_tot=TA_tot, TB_tot=TB_tot, T_tot=T_tot,
                dstloc=dstloc, idxA=idxA, idxB=idxB, cnt=cnt)


def _wrap16(idx_flat):
    """Logical int16 index list [n*P] -> [128, n*8] (16-wrap, replicated 8x)."""
    n = idx_flat.shape[0]
    assert n % 16 == 0
    arr = np.empty((16, n // 16), dtype=np.int16)
    j = np.arange(n)
    arr[j % 16, j // 16] = idx_flat
    return np.tile(arr, (8, 1))


def _build_nc(plan):
    _mode = _os.environ.get("KERNEL_MODE", "full")
    T_tot = plan["T_tot"]
    T_pad2 = (T_tot + 1) // 2 * 2       # dstloc bf16 cols padded to fp32 pairs
    sched = plan["sched"]
    # idx tensor: per-batch [A idx | B idx] regions; split so the first
    # batches' indices land early and gathers start ASAP
    ci_tot = sched[-1]["ci_off"] + (sched[-1]["ka"] + sched[-1]["kb"]) * 4
    n_split = min(2, len(sched))
    ci_split = sched[n_split]["ci_off"] if n_split < len(sched) else ci_tot
    # const fp32 column layout (everything the gathers do NOT need)
    c_dst = 0                           # dstloc bf16 [128, T_pad2]
    c_iota = T_pad2 // 2                # iota bf16 [128, 128] -> 64 f32 cols
    c_wt = c_iota + 64                  # W^T bf16 [128, 128] -> 64 f32 cols
    c_brow = c_wt + 64                  # bias bf16 row [1, 128] -> 64 f32 cols
    c_tot = c_brow + 64
    plan["c_layout"] = (c_dst, c_iota, c_wt, c_brow, c_tot, ci_tot, T_pad2)

    nc = bacc.Bacc("TRN2", num_swdge_queues=NQ)
    featA = nc.declare_dram_parameter("featA", [VHALF, P], BF16, isOutput=False)
    featB = nc.declare_dram_parameter("featB", [N_NODES - VHALF, P], BF16,
                                      isOutput=False)
    idxs = nc.declare_dram_parameter("idxs", [P, ci_tot], F32, isOutput=False)
    consts = nc.declare_dram_parameter("consts", [P, c_tot], F32, isOutput=False)
    out = nc.declare_dram_parameter("out", [W_SLOTS * P, P], F32, isOutput=True)

    with tile.TileContext(nc) as tc:
        with (
            tc.tile_pool(name="const", bufs=1) as const_pool,
            tc.tile_pool(name="work", bufs=3) as work_pool,
            tc.tile_pool(name="outp", bufs=3) as out_pool,
            tc.tile_pool(name="psum", bufs=4, space="PSUM") as psum_pool,
        ):
            ci0 = const_pool.tile([P, ci_split], F32)
            nc.sync.dma_start(out=ci0[:], in_=idxs[:, 0:ci_split])
            cs = const_pool.tile([P, c_tot], F32)
            nc.scalar.dma_start(out=cs[:], in_=consts[:])
            ci1 = None
            if ci_split < ci_tot:
                ci1 = const_pool.tile([P, ci_tot - ci_split], F32)
                # WAW chain: ci1's bulk DMA must follow this copy (which reads
                # ci0), so it doesn't compete with ci0's transfer for DMA
                # engines while the first gathers wait on ci0
                nc.vector.tensor_copy(out=ci1[0:1, 0:1], in_=ci0[0:1, 0:1])
                nc.scalar.dma_start(out=ci1[:], in_=idxs[:, ci_split:ci_tot])
            dst_bf = cs[:, c_dst:c_dst + T_pad2 // 2].bitcast(BF16)
            iota_bf = cs[:, c_iota:c_iota + 64].bitcast(BF16)
            wt_bf = cs[:, c_wt:c_wt + 64].bitcast(BF16)
            brow_bf = cs[0:1, c_brow:c_brow + 64].bitcast(BF16)
            ones_bf = const_pool.tile([1, P], BF16)
            nc.vector.memset(ones_bf[:], 1.0)

            # warm-up gather: forces the SWDGE Q7 library load to overlap the
            # idx-table DMAs instead of delaying the first real gather
            warm_idx = const_pool.tile([P, 8], I16)
            nc.vector.memset(warm_idx[:], 0)
            warm_out = const_pool.tile([P, 1, P], BF16)
            nc.gpsimd.dma_gather(
                out_ap=warm_out[:], in_ap=featA[:], idxs_ap=warm_idx[:],
                num_idxs=P, num_idxs_reg=P, elem_size=P, queue_num=0,
            )

            # zero the 3 msgs ring buffers once (idle DVE, overlaps startup):
            # gathers then skip trailing -1 idx everywhere and the stale
            # regions read as finite 0.0 under zero one-hot columns
            kmax = max(b["ka"] + b["kb"] for b in sched)
            for _ in range(3):
                probe = work_pool.tile([P, kmax, P], BF16, tag="msgs")
                nc.vector.memset(probe[:], 0.0)

            KA, KB = plan["KA"], plan["KB"]
            cStatA, cStatB = plan["cStatA"], plan["cStatB"]
            gq = [0]
            _trim = _os.environ.get("KTRIM", "1") != "0"
            _regs = {}

            def _reg_for(v):
                # hoist count registers so each gather is a single Pool-queue
                # instruction (no per-call MOVE halving the exec-queue depth)
                if v not in _regs:
                    _regs[v] = nc.gpsimd.to_reg(v)
                return _regs[v]

            for bi, bt in enumerate(sched):
                ka_b, kb_b = bt["ka"], bt["kb"]
                k_b = ka_b + kb_b
                msgs = work_pool.tile([P, k_b, P], BF16, tag="msgs")
                if bi < n_split:
                    reg = ci0[:, bt["ci_off"]:bt["ci_off"] + k_b * 4]
                else:
                    o = bt["ci_off"] - ci_split
                    reg = ci1[:, o:o + k_b * 4]
                regA = reg[:, 0:ka_b * 4].bitcast(I16)
                regB = reg[:, ka_b * 4:k_b * 4].bitcast(I16)

                def _win_calls(src_tab, reg16, out_c0, idx_t0, ntiles, creal,
                               full_pad, msgs=msgs):
                    """Gather calls for one window-half: split at 8 tiles;
                    trailing -1 idx (beyond creal) is skipped unless
                    full_pad (buffer-init batches gather the zero padding)."""
                    for off in range(0, ntiles, 8):
                        nk = min(8, ntiles - off)
                        if full_pad or not _trim:
                            rcall = nk * P
                        else:
                            rcall = max(0, min(creal - off * P, nk * P))
                            if rcall == 0:
                                continue
                        if _os.environ.get("QPAIR"):
                            qn = (gq[0] // 2) % NQ
                        else:
                            qn = gq[0] % NQ
                        nc.gpsimd.dma_gather(
                            out_ap=msgs[:, out_c0 + off:out_c0 + off + nk, :],
                            in_ap=src_tab,
                            idxs_ap=reg16[:, (idx_t0 + off) * 8:
                                          (idx_t0 + off + nk) * 8],
                            num_idxs=nk * P,
                            num_idxs_reg=_reg_for(rcall) if not _trim else rcall,
                            elem_size=P,
                            single_packet=not _os.environ.get("MULTIPKT"),
                            queue_num=qn,
                        )
                        gq[0] += 1

                if _mode != "nogather":
                    full_pad = bi < INIT_BATCHES
                    for w, cols in bt["wins"]:
                        nA = int(KA[w])
                        if nA:
                            _win_calls(featA[:], regA, cols[0], cols[0], nA,
                                       int(cStatA[w]), full_pad)
                        if int(KB[w]):
                            bcol0 = cols[nA]
                            _win_calls(featB[:], regB, bcol0, bcol0 - ka_b,
                                       int(KB[w]), int(cStatB[w]), full_pad)
                if _mode == "gatheronly":
                    continue
                onehot = work_pool.tile([P, k_b, P], BF16, tag="onehot")
                for o in range(0, k_b, 16):
                    kk = min(16, k_b - o)
                    nc.vector.tensor_tensor(
                        out=onehot[:, o:o + kk, :],
                        in0=dst_bf[:, bt["t_base"] + o:bt["t_base"] + o + kk]
                            .to_broadcast([P, kk, P]),
                        in1=dataclasses.replace(
                            iota_bf, ap=[iota_bf.ap[0], [0, kk], iota_bf.ap[1]]),
                        op=mybir.AluOpType.is_equal,
                    )
                for w, cols in bt["wins"]:
                    aggT_ps = psum_pool.tile([P, P], F32, tag="aggT")
                    for i, ccol in enumerate(cols):
                        nc.tensor.matmul(
                            out=aggT_ps[:],
                            lhsT=msgs[:, ccol, :],
                            rhs=onehot[:, ccol, :],
                            start=(i == 0),
                            stop=(i == len(cols) - 1),
                        )
                    aggT_sb = out_pool.tile([P, P], BF16, tag="aggT_sb")
                    nc.vector.tensor_copy(out=aggT_sb[:], in_=aggT_ps[:])
                    out2_ps = psum_pool.tile([P, P], F32, tag="out2")
                    nc.tensor.matmul(out=out2_ps[:], lhsT=aggT_sb[:], rhs=wt_bf,
                                     start=True, stop=False)
                    nc.tensor.matmul(out=out2_ps[:], lhsT=ones_bf[:], rhs=brow_bf,
                                     start=False, stop=True)
                    out_sb = out_pool.tile([P, P], F32, tag="out_sb")
                    nc.scalar.activation(out=out_sb[:], in_=out2_ps[:],
                                         func=mybir.ActivationFunctionType.Relu)
                    nc.sync.dma_start(out=out[w * P:(w + 1) * P, :], in_=out_sb[:])
    nc.finalize()
    return nc


_CACHE = {}


def _prepare(feature, src, dst, W, b):
    feature = np.asarray(feature, dtype=np.float32)
    W = np.asarray(W, dtype=np.float32)
    b = np.asarray(b, dtype=np.float32)
    key = (hash(np.asarray(src).tobytes()), hash(np.asarray(dst).tobytes()))
    if key not in _CACHE:
        plan = _make_plan(src, dst)
        nc = _build_nc(plan)
        _CACHE.clear()
        _CACHE[key] = (plan, nc)
    plan, nc = _CACHE[key]
    c_dst, c_iota, c_wt, c_brow, c_tot, ci_tot, T_pad2 = plan["c_layout"]
    iota = np.arange(P, dtype=BF)
    in_maps = []
    feat_bf = feature.astype(BF)
    featA = np.ascontiguousarray(feat_bf[:VHALF])
    featB = np.ascontiguousarray(feat_bf[VHALF:])
    dst_pad = np.full((NC, P, T_pad2), -1.0, dtype=BF)
    dst_pad[:, :, :plan["T_tot"]] = plan["dstloc"]
    for c in range(NC):
        idxs = np.zeros((P, ci_tot), dtype=np.float32)
        for bt in plan["sched"]:
            o = bt["ci_off"]
            ka, kb = bt["ka"], bt["kb"]
            if ka:
                sl = plan["idxA"][c][bt["a_base"] * P:(bt["a_base"] + ka) * P]
                idxs[:, o:o + ka * 4] = _wrap16(sl).view(np.float32)
            if kb:
                sl = plan["idxB"][c][bt["b_base"] * P:(bt["b_base"] + kb) * P]
                idxs[:, o + ka * 4:o + (ka + kb) * 4] = \
                    _wrap16(sl).view(np.float32)
        consts = np.zeros((P, c_tot), dtype=np.float32)
        consts[:, c_dst:c_dst + T_pad2 // 2] = dst_pad[c].view(np.float32)
        consts[:, c_iota:c_iota + 64] = \
            np.broadcast_to(iota[None, :], (P, P)).copy().view(np.float32)
        consts[:, c_wt:c_wt + 64] = \
            np.ascontiguousarray(W.T.astype(BF)).view(np.float32)
        consts[0, c_brow:c_brow + 64] = b.astype(BF).view(np.float32)
        in_maps.append({"featA": featA, "featB": featB, "idxs": idxs,
                        "consts": consts})
    return plan, nc, in_maps


def _assemble(plan, results):
    out_full = np.zeros((N_NODES, P), dtype=np.float32)
    assign = plan["assign"]
    for c in range(NC):
        oc = results[c]["out"]
        for w in range(W_SLOTS):
            g = assign[c, w]
            if g < 0:
                continue
            n0 = int(g) << 7
            n1 = min(n0 + P, N_NODES)
            out_full[n0:n1] = oc[w * P:w * P + (n1 - n0)]
    return out_full


def kernel(feature, src, dst, W, b):
    plan, nc, in_maps = _prepare(feature, src, dst, W, b)
    res = run_bass_kernel_spmd(nc, in_maps, list(range(NC)))
    return _assemble(plan, res.results)


def kernel_traced(feature, src, dst, W, b, **trace_kwargs):
    """Like kernel() but returns (output, BassKernelResults) with trace."""
    plan, nc, in_maps = _prepare(feature, src, dst, W, b)
    res = run_bass_kernel_spmd(nc, in_maps, list(range(NC)), trace=True,
                               **trace_kwargs)
    return _assemble(plan, res.results), res


# revision 31
# speedup vs baseline: 1.2004x; 1.2004x over previous
"""GNN message passing (copy_src + segment_sum + Linear + ReLU) on 8 TRN2 cores.

Strategy (sharding_hint): partition dst nodes into 128-node windows; assign
windows to the 8 cores (balanced by edge-tile count, serpentine over windows
sorted by cost). Each core holds a full replica of the (split) feature table
in its HBM in bf16 and gathers the src rows of its edges with dma_gather (the
int16 index range forces a two-half table split at 25000). Aggregation happens
on-chip: for each 128-edge tile, a one-hot(dst_local) matrix is built with a
DVE is_equal against an iota row (all bf16), and PE matmuls accumulate
aggT[f, n] += msgs[e, f].T @ onehot[e, n] into fp32 PSUM per window. The node
update (Linear + bias + ReLU) runs per window: transform matmul with W^T
(bf16), K=1 ones-row matmul to add the bias, ACT ReLU, DMA out in fp32.
No collectives — the host splits edges and concatenates the per-core window
outputs. Gathers rotate over 4 SWDGE queues so descriptor-ring drain overlaps
generation.

Self-contained: shapes hardcoded for feature[50000,128], src/dst[640000],
W[128,128], b[128].
"""
import dataclasses
import os as _os

import ml_dtypes
import numpy as np

import concourse.bacc as bacc
import concourse.bass as bass
import concourse.tile as tile
from concourse import mybir
from concourse.bass_utils import run_bass_kernel_spmd

P = 128
N_NODES = 50000
N_EDGES = 640000
VHALF = 25000          # feature table split point (int16 gather index range)
NC = 8
G = (N_NODES + P - 1) // P          # 391 global windows
W_SLOTS = (G + NC - 1) // NC        # 49 window slots per core
BATCH_TILES = 64                    # max edge tiles per gather batch
NQ = 4                              # SWDGE queues for gather rotation
INIT_BATCHES = 0                    # msgs buffers are pre-memset; no init pads

F32 = mybir.dt.float32
BF16 = mybir.dt.bfloat16
I16 = mybir.dt.int16
BF = ml_dtypes.bfloat16


def _make_plan(src, dst):
    """Host-side partition of edges into (core, slot, tile) with static,
    core-uniform tile counts. Returns the static schedule + per-core arrays."""
    src = np.asarray(src, dtype=np.int64)
    dst = np.asarray(dst, dtype=np.int64)
    win = dst >> 7
    order = np.lexsort((src, win))          # by window, then src (HBM locality)
    src_s = src[order]
    dst_s = dst[order]
    win_s = win[order]

    cnt = np.bincount(win_s, minlength=G)
    cntA = np.bincount(win_s[src_s < VHALF], minlength=G)
    cntB = cnt - cntA
    start = np.concatenate([[0], np.cumsum(cnt)])[:G]
    tilesA = -(-cntA // P)
    tilesB = -(-cntB // P)

    # group windows into slots of 8 with matching (tilesA, tilesB) so the
    # per-slot max over cores (the static padded capacity) is tight; within a
    # slot, hand the biggest window to the least-loaded core.
    idx_sorted = sorted(range(G), key=lambda g: (-tilesA[g], -tilesB[g]))
    assign = -np.ones((NC, W_SLOTS), dtype=np.int64)
    core_load = np.zeros(NC, dtype=np.int64)
    for w in range(W_SLOTS):
        chunk = idx_sorted[w * NC:(w + 1) * NC]
        chunk = sorted(chunk, key=lambda g: -cnt[g])
        order = np.argsort(core_load, kind="stable")
        for g, c in zip(chunk, order):
            assign[c, w] = g
            core_load[c] += cnt[g]

    # static per-slot tile capacities and real index counts (shared across
    # cores; cStat = max real edge count over cores, so num_idxs_reg can be
    # core-uniform while trailing -1 padding is skipped by the SWDGE)
    KA = np.zeros(W_SLOTS, dtype=np.int64)
    KB = np.zeros(W_SLOTS, dtype=np.int64)
    cStatA = np.zeros(W_SLOTS, dtype=np.int64)
    cStatB = np.zeros(W_SLOTS, dtype=np.int64)
    for w in range(W_SLOTS):
        for c in range(NC):
            g = assign[c, w]
            if g >= 0:
                cStatA[w] = max(cStatA[w], cntA[g])
                cStatB[w] = max(cStatB[w], cntB[g])
        KA[w] = -(-cStatA[w] // P)
        KB[w] = -(-cStatB[w] // P)
        if KA[w] + KB[w] == 0:
            KA[w] = 1                      # dummy tile keeps PSUM initialized

    # greedy batches of slots, <= BATCH_TILES tiles each; the final slots get
    # small batches so the post-gather compute tail is short
    slot_tiles = [int(KA[w] + KB[w]) for w in range(W_SLOTS)]
    total_tiles = sum(slot_tiles)
    batches = []
    cur = []
    cur_tiles = 0
    done_tiles = 0
    for w in range(W_SLOTS):
        kw = slot_tiles[w]
        if (total_tiles - done_tiles) <= 48:
            cap = 16        # small final batches: short post-gather tail
        else:
            cap = BATCH_TILES
        if cur and cur_tiles + kw > cap:
            batches.append(cur)
            cur = []
            cur_tiles = 0
        cur.append(w)
        cur_tiles += kw
        done_tiles += kw
    if cur:
        batches.append(cur)

    TA_tot = int(KA.sum())
    TB_tot = int(KB.sum())
    T_tot = TA_tot + TB_tot

    # static schedule description (identical across cores); ci_off is the
    # f32-column offset of this batch's [A idx | B idx] region in the idx
    # tensor (per-batch packing so the idx DMA can be split for early start)
    sched = []
    t_base = 0
    a_base = 0
    b_base = 0
    ci_off = 0
    for slots in batches:
        ka_b = int(sum(KA[w] for w in slots))
        kb_b = int(sum(KB[w] for w in slots))
        wins = []
        pa = 0
        pb = 0
        for w in slots:
            cols = list(range(pa, pa + int(KA[w]))) + \
                   list(range(ka_b + pb, ka_b + pb + int(KB[w])))
            wins.append((w, cols))
            pa += int(KA[w])
            pb += int(KB[w])
        sched.append(dict(ka=ka_b, kb=kb_b, t_base=t_base,
                          a_base=a_base, b_base=b_base, ci_off=ci_off,
                          wins=wins))
        t_base += ka_b + kb_b
        a_base += ka_b
        b_base += kb_b
        ci_off += (ka_b + kb_b) * 4

    # per-core data arrays; batches >= INIT_BATCHES get trailing -1 index
    # padding per window-half (SWDGE skips it), earlier batches keep full
    # zero-padding so every msgs buffer region gets initialized once
    dstloc = np.full((NC, P, T_tot), -1.0, dtype=BF)
    idxA = np.zeros((NC, TA_tot * P), dtype=np.int16)
    idxB = np.zeros((NC, TB_tot * P), dtype=np.int16)
    for bi, bt in enumerate(sched):
        ka_b = bt["ka"]
        for w, cols in bt["wins"]:
            nA = len([c for c in cols if c < ka_b])
            for c in range(NC):
                g = assign[c, w]
                if g < 0:
                    continue
                e0 = start[g]
                ca, cb = int(cntA[g]), int(cntB[g])
                dl = (dst_s[e0:e0 + ca + cb] - (g << 7)).astype(BF)
                sv = src_s[e0:e0 + ca + cb]
                # A half
                if ca:
                    jpos = np.arange(ca)
                    dstloc[c, jpos % P, bt["t_base"] + cols[0] + jpos // P] = dl[:ca]
                    # logical gather position = (prefixA + tile)*P + lane
                    gpos = (bt["a_base"] + cols[0]) * P + jpos
                    idxA[c, gpos] = sv[:ca].astype(np.int16)
                # B half
                if cb:
                    jpos = np.arange(cb)
                    bcol0 = cols[nA] if cb else 0
                    dstloc[c, jpos % P, bt["t_base"] + bcol0 + jpos // P] = dl[ca:]
                    gpos = (bt["b_base"] + (bcol0 - ka_b)) * P + jpos
                    idxB[c, gpos] = (sv[ca:] - VHALF).astype(np.int16)
            if bi >= INIT_BATCHES and _os.environ.get("KTRIM", "1") != "0":
                if KA[w]:
                    p0 = (bt["a_base"] + cols[0]) * P
                    idxA[:, p0 + cStatA[w]:p0 + int(KA[w]) * P] = -1
                if KB[w]:
                    bcol0 = cols[nA]
                    p0 = (bt["b_base"] + (bcol0 - ka_b)) * P
                    idxB[:, p0 + cStatB[w]:p0 + int(KB[w]) * P] = -1

    return dict(sched=sched, assign=assign, KA=KA, KB=KB,
                cStatA=cStatA, cStatB=cStatB,
                TA_tot=TA_tot, TB_tot=TB_tot, T_tot=T_tot,
                dstloc=dstloc, idxA=idxA, idxB=idxB, cnt=cnt)


def _wrap16(idx_flat):
    """Logical int16 index list [n*P] -> [128, n*8] (16-wrap, replicated 8x)."""
    n = idx_flat.shape[0]
    assert n % 16 == 0
    arr = np.empty((16, n // 16), dtype=np.int16)
    j = np.arange(n)
    arr[j % 16, j // 16] = idx_flat
    return np.tile(arr, (8, 1))


def _build_nc(plan):
    _mode = _os.environ.get("KERNEL_MODE", "full")
    T_tot = plan["T_tot"]
    T_pad2 = (T_tot + 1) // 2 * 2       # dstloc bf16 cols padded to fp32 pairs
    sched = plan["sched"]
    # idx tensor: per-batch [A idx | B idx] regions; split so the first
    # batches' indices land early and gathers start ASAP
    ci_tot = sched[-1]["ci_off"] + (sched[-1]["ka"] + sched[-1]["kb"]) * 4
    n_split = min(2, len(sched))
    ci_split = sched[n_split]["ci_off"] if n_split < len(sched) else ci_tot
    # const fp32 column layout (everything the gathers do NOT need)
    c_dst = 0                           # dstloc bf16 [128, T_pad2]
    c_iota = T_pad2 // 2                # iota bf16 [128, 128] -> 64 f32 cols
    c_wt = c_iota + 64                  # W^T bf16 [128, 128] -> 64 f32 cols
    c_bcol = c_wt + 64                  # bias f32 column [128, 1]
    c_tot = c_bcol + 1
    plan["c_layout"] = (c_dst, c_iota, c_wt, c_bcol, c_tot, ci_tot, T_pad2)

    nc = bacc.Bacc("TRN2", num_swdge_queues=NQ)
    featA = nc.declare_dram_parameter("featA", [VHALF, P], BF16, isOutput=False)
    featB = nc.declare_dram_parameter("featB", [N_NODES - VHALF, P], BF16,
                                      isOutput=False)
    idxs = nc.declare_dram_parameter("idxs", [P, ci_tot], F32, isOutput=False)
    consts = nc.declare_dram_parameter("consts", [P, c_tot], F32, isOutput=False)
    out = nc.declare_dram_parameter("out", [W_SLOTS * P, P], F32, isOutput=True)

    with tile.TileContext(nc) as tc:
        with (
            tc.tile_pool(name="const", bufs=1) as const_pool,
            tc.tile_pool(name="work", bufs=3) as work_pool,
            tc.tile_pool(name="outp", bufs=3) as out_pool,
            tc.tile_pool(name="psum", bufs=4, space="PSUM") as psum_pool,
        ):
            ci0 = const_pool.tile([P, ci_split], F32)
            nc.sync.dma_start(out=ci0[:], in_=idxs[:, 0:ci_split])
            cs = const_pool.tile([P, c_tot], F32)
            nc.scalar.dma_start(out=cs[:], in_=consts[:])
            ci1 = None
            if ci_split < ci_tot:
                ci1 = const_pool.tile([P, ci_tot - ci_split], F32)
                # WAW chain: ci1's bulk DMA must follow this copy (which reads
                # ci0), so it doesn't compete with ci0's transfer for DMA
                # engines while the first gathers wait on ci0
                nc.vector.tensor_copy(out=ci1[0:1, 0:1], in_=ci0[0:1, 0:1])
                nc.scalar.dma_start(out=ci1[:], in_=idxs[:, ci_split:ci_tot])
            dst_bf = cs[:, c_dst:c_dst + T_pad2 // 2].bitcast(BF16)
            iota_bf = cs[:, c_iota:c_iota + 64].bitcast(BF16)
            wt_bf = cs[:, c_wt:c_wt + 64].bitcast(BF16)
            bcol = cs[:, c_bcol:c_bcol + 1]

            # warm-up gather: forces the SWDGE Q7 library load to overlap the
            # idx-table DMAs instead of delaying the first real gather
            warm_idx = const_pool.tile([P, 8], I16)
            nc.vector.memset(warm_idx[:], 0)
            warm_out = const_pool.tile([P, 1, P], BF16)
            nc.gpsimd.dma_gather(
                out_ap=warm_out[:], in_ap=featA[:], idxs_ap=warm_idx[:],
                num_idxs=P, num_idxs_reg=P, elem_size=P, queue_num=0,
            )

            # zero the 3 msgs ring buffers once (idle DVE, overlaps startup):
            # gathers then skip trailing -1 idx everywhere and the stale
            # regions read as finite 0.0 under zero one-hot columns
            kmax = max(b["ka"] + b["kb"] for b in sched)
            for _ in range(3):
                probe = work_pool.tile([P, kmax, P], BF16, tag="msgs")
                nc.vector.memset(probe[:], 0.0)

            KA, KB = plan["KA"], plan["KB"]
            cStatA, cStatB = plan["cStatA"], plan["cStatB"]
            gq = [0]
            _trim = _os.environ.get("KTRIM", "1") != "0"
            _regs = {}

            def _reg_for(v):
                # hoist count registers so each gather is a single Pool-queue
                # instruction (no per-call MOVE halving the exec-queue depth)
                if v not in _regs:
                    _regs[v] = nc.gpsimd.to_reg(v)
                return _regs[v]

            for bi, bt in enumerate(sched):
                ka_b, kb_b = bt["ka"], bt["kb"]
                k_b = ka_b + kb_b
                msgs = work_pool.tile([P, k_b, P], BF16, tag="msgs")
                if bi < n_split:
                    reg = ci0[:, bt["ci_off"]:bt["ci_off"] + k_b * 4]
                else:
                    o = bt["ci_off"] - ci_split
                    reg = ci1[:, o:o + k_b * 4]
                regA = reg[:, 0:ka_b * 4].bitcast(I16)
                regB = reg[:, ka_b * 4:k_b * 4].bitcast(I16)

                def _win_calls(src_tab, reg16, out_c0, idx_t0, ntiles, creal,
                               full_pad, msgs=msgs):
                    """Gather calls for one window-half: split at 8 tiles;
                    trailing -1 idx (beyond creal) is skipped unless
                    full_pad (buffer-init batches gather the zero padding)."""
                    for off in range(0, ntiles, 8):
                        nk = min(8, ntiles - off)
                        if full_pad or not _trim:
                            rcall = nk * P
                        else:
                            rcall = max(0, min(creal - off * P, nk * P))
                            if rcall == 0:
                                continue
                        if _os.environ.get("QPAIR"):
                            qn = (gq[0] // 2) % NQ
                        else:
                            qn = gq[0] % NQ
                        nc.gpsimd.dma_gather(
                            out_ap=msgs[:, out_c0 + off:out_c0 + off + nk, :],
                            in_ap=src_tab,
                            idxs_ap=reg16[:, (idx_t0 + off) * 8:
                                          (idx_t0 + off + nk) * 8],
                            num_idxs=nk * P,
                            num_idxs_reg=_reg_for(rcall) if not _trim else rcall,
                            elem_size=P,
                            single_packet=not _os.environ.get("MULTIPKT"),
                            queue_num=qn,
                        )
                        gq[0] += 1

                if _mode != "nogather":
                    full_pad = bi < INIT_BATCHES
                    for w, cols in bt["wins"]:
                        nA = int(KA[w])
                        if nA:
                            _win_calls(featA[:], regA, cols[0], cols[0], nA,
                                       int(cStatA[w]), full_pad)
                        if int(KB[w]):
                            bcol0 = cols[nA]
                            _win_calls(featB[:], regB, bcol0, bcol0 - ka_b,
                                       int(KB[w]), int(cStatB[w]), full_pad)
                if _mode == "gatheronly":
                    continue
                onehot = work_pool.tile([P, k_b, P], BF16, tag="onehot")
                for o in range(0, k_b, 16):
                    kk = min(16, k_b - o)
                    nc.vector.tensor_tensor(
                        out=onehot[:, o:o + kk, :],
                        in0=dst_bf[:, bt["t_base"] + o:bt["t_base"] + o + kk]
                            .to_broadcast([P, kk, P]),
                        in1=dataclasses.replace(
                            iota_bf, ap=[iota_bf.ap[0], [0, kk], iota_bf.ap[1]]),
                        op=mybir.AluOpType.is_equal,
                    )
                for w, cols in bt["wins"]:
                    aggT_ps = psum_pool.tile([P, P], F32, tag="aggT")
                    for i, ccol in enumerate(cols):
                        nc.tensor.matmul(
                            out=aggT_ps[:],
                            lhsT=msgs[:, ccol, :],
                            rhs=onehot[:, ccol, :],
                            start=(i == 0),
                            stop=(i == len(cols) - 1),
                        )
                    aggT_sb = out_pool.tile([P, P], BF16, tag="aggT_sb")
                    nc.vector.tensor_copy(out=aggT_sb[:], in_=aggT_ps[:])
                    # transform with stationary W^T: out2[o, n] = (W @ agg)[o, n]
                    # (transposed; host transposes back); bias + ReLU fused on
                    # the ACT engine (bias is per-partition in this layout)
                    out2_ps = psum_pool.tile([P, P], F32, tag="out2")
                    nc.tensor.matmul(out=out2_ps[:], lhsT=wt_bf, rhs=aggT_sb[:],
                                     start=True, stop=True)
                    out_sb = out_pool.tile([P, P], F32, tag="out_sb")
                    nc.scalar.activation(out=out_sb[:], in_=out2_ps[:],
                                         func=mybir.ActivationFunctionType.Relu,
                                         bias=bcol)
                    nc.sync.dma_start(out=out[w * P:(w + 1) * P, :], in_=out_sb[:])
    nc.finalize()
    return nc


_CACHE = {}


def _prepare(feature, src, dst, W, b):
    feature = np.asarray(feature, dtype=np.float32)
    W = np.asarray(W, dtype=np.float32)
    b = np.asarray(b, dtype=np.float32)
    key = (hash(np.asarray(src).tobytes()), hash(np.asarray(dst).tobytes()))
    if key not in _CACHE:
        plan = _make_plan(src, dst)
        nc = _build_nc(plan)
        _CACHE.clear()
        _CACHE[key] = (plan, nc)
    plan, nc = _CACHE[key]
    c_dst, c_iota, c_wt, c_bcol, c_tot, ci_tot, T_pad2 = plan["c_layout"]
    iota = np.arange(P, dtype=BF)
    in_maps = []
    feat_bf = feature.astype(BF)
    featA = np.ascontiguousarray(feat_bf[:VHALF])
    featB = np.ascontiguousarray(feat_bf[VHALF:])
    dst_pad = np.full((NC, P, T_pad2), -1.0, dtype=BF)
    dst_pad[:, :, :plan["T_tot"]] = plan["dstloc"]
    for c in range(NC):
        idxs = np.zeros((P, ci_tot), dtype=np.float32)
        for bt in plan["sched"]:
            o = bt["ci_off"]
            ka, kb = bt["ka"], bt["kb"]
            if ka:
                sl = plan["idxA"][c][bt["a_base"] * P:(bt["a_base"] + ka) * P]
                idxs[:, o:o + ka * 4] = _wrap16(sl).view(np.float32)
            if kb:
                sl = plan["idxB"][c][bt["b_base"] * P:(bt["b_base"] + kb) * P]
                idxs[:, o + ka * 4:o + (ka + kb) * 4] = \
                    _wrap16(sl).view(np.float32)
        consts = np.zeros((P, c_tot), dtype=np.float32)
        consts[:, c_dst:c_dst + T_pad2 // 2] = dst_pad[c].view(np.float32)
        consts[:, c_iota:c_iota + 64] = \
            np.broadcast_to(iota[None, :], (P, P)).copy().view(np.float32)
        consts[:, c_wt:c_wt + 64] = \
            np.ascontiguousarray(W.T.astype(BF)).view(np.float32)
        consts[:, c_bcol] = b
        in_maps.append({"featA": featA, "featB": featB, "idxs": idxs,
                        "consts": consts})
    return plan, nc, in_maps


def _assemble(plan, results):
    out_full = np.zeros((N_NODES, P), dtype=np.float32)
    assign = plan["assign"]
    for c in range(NC):
        oc = results[c]["out"]
        for w in range(W_SLOTS):
            g = assign[c, w]
            if g < 0:
                continue
            n0 = int(g) << 7
            n1 = min(n0 + P, N_NODES)
            # device blocks are [out_fea, node] (transposed transform)
            out_full[n0:n1] = oc[w * P:(w + 1) * P, :n1 - n0].T
    return out_full


def kernel(feature, src, dst, W, b):
    plan, nc, in_maps = _prepare(feature, src, dst, W, b)
    res = run_bass_kernel_spmd(nc, in_maps, list(range(NC)))
    return _assemble(plan, res.results)


def kernel_traced(feature, src, dst, W, b, **trace_kwargs):
    """Like kernel() but returns (output, BassKernelResults) with trace."""
    plan, nc, in_maps = _prepare(feature, src, dst, W, b)
    res = run_bass_kernel_spmd(nc, in_maps, list(range(NC)), trace=True,
                               **trace_kwargs)
    return _assemble(plan, res.results), res
